# revision 38
# baseline (speedup 1.0000x reference)
"""Trainium2 Bass kernel for nn_PatternBranch (conv3x3/s2+relu -> routed heads).

Strategy
--------
Everything after the conv folds into ONE matmul: with feats0 flattened as
(position p, channel co), the base head, the pattern head (channel-gather
scatter-added over pattern_set_index), and the match head (GAP dot match_w
== sum over (p, co) of feats0 * match_w[co]/1024) concatenate into a single
fused weight W5[p, co, 0:5].  The device then computes, per core:

    conv (im2col K=28 matmul, bias folded in as a ones-row)  ->  PSUM
    -> relu-evict (ACT+DVE+POOL 3-way split) -> feats0 in SBUF
    -> fused matmul  ->  partial logits [5, 256]

Sharding: the 1024 output positions are split 8 ways (128 positions /
core = 4 rows of the 32x32 output grid); every core sees all 256 samples.
Each fused matmul packs TWO positions (M=10, N=512 sample-columns; the
block-cross terms land in ignored PSUM quadrants), and W5 shards cleanly.
Host sums the 8 partial logits and runs the tiny [256,5] epilogue
(sigmoid/softmax/route).

Matmul inputs are bf16 (PE streams 1 col/cycle at 2.4 GHz warm; fp32/fp32r
paths cap at ~1.2 GHz effective).  PSUM accumulation is fp32.  Conv matmuls
are row-tiled (K=28 strips at partitions 32q) so 4 run concurrently; the
conv bias rides along as im2col row 27 = 1.0 with cw row 27 = conv_b, so
the PSUM->SBUF eviction is a bare relu and can rotate over ScalarE,
VectorE and PoolE.  A handful of short dummy matmuls open the HAM clock
gate while the first DMA chunks land; the input DMA is chunked so the
first conv step waits only on the first 1024 columns of each quarter, and
the fused matmuls trail their evictions by two steps so the PE never
stalls.
"""
import sys

for _p in ("/opt/trn_rl_repo", "/root/.axon_site/_ro/trn_rl_repo"):
    if _p not in sys.path:
        sys.path.append(_p)

import numpy as np
import ml_dtypes

import concourse.bacc as bacc
import concourse.mybir as mybir
import concourse.tile as tile
from concourse.bass_utils import run_bass_kernel_spmd

F32 = mybir.dt.float32
BF16 = mybir.dt.bfloat16
NP_BF16 = ml_dtypes.bfloat16

B = 256          # batch
HW_IN = 64       # input spatial
CIN = 3
COUT = 128
KPAT = 32        # gathered channels for pattern head
P_GRID = 32      # output spatial (stride 2)
N_CORES = 8
P_CORE = 128     # positions per core (4 rows of 32)
QB = 64          # samples per quarter
NQ = 4           # quarters (4 x 64 = 256 samples)
KC = 27          # im2col contraction (3*3*3)
KC2 = 28         # + ones-row carrying the conv bias
NT = 16          # time steps: 8 positions x 4 quarters each

_NC_CACHE = {}


def _build_nc():
    """One SPMD program, same for all 8 cores."""
    nc = bacc.Bacc("TRN2", target_bir_lowering=False, debug=False)

    imcol = nc.dram_tensor("imcol", [NQ, KC2, P_CORE * QB], BF16,
                           kind="ExternalInput")
    cw4 = nc.dram_tensor("cw4", [128, COUT], BF16, kind="ExternalInput")
    w5 = nc.dram_tensor("w5", [COUT, P_CORE * 5], BF16, kind="ExternalInput")
    outp = nc.dram_tensor("out", [10, 2 * B], F32, kind="ExternalOutput")

    with tile.TileContext(nc) as tc:
        with tc.tile_pool(name="singles", bufs=1) as singles, \
             tc.tile_pool(name="convps", bufs=2, space="PSUM") as convps, \
             tc.tile_pool(name="convq3", bufs=1, space="PSUM") as convq3, \
             tc.tile_pool(name="faccps", bufs=1, space="PSUM") as faccps:

            cw_sb = singles.tile([128, COUT], BF16)
            imq = singles.tile([128, P_CORE * QB], BF16)
            w5_sb = singles.tile([COUT, P_CORE * 5], BF16)
            actwarm = singles.tile([128, 1], F32)
            zdummy = singles.tile([128, 384], BF16)
            # feats0[co, q, p, b]  (position-major within quarter)
            feats0 = singles.tile([COUT, NQ, P_CORE, QB], BF16)
            out_sb = singles.tile([10, 2 * B], F32)

            facc0 = faccps.tile([128, 2 * B], F32)

            def imq_dma(eng, q, lo, hi):
                eng.dma_start(
                    out=imq[32 * q:32 * q + KC2, lo:hi],
                    in_=imcol[q, :, lo:hi])

            # Vector memsets first so the dummy matmuls and ACT table
            # preload are unblocked immediately.
            nc.vector.memset(actwarm[:, :], 0.0)
            nc.vector.memset(zdummy[:, :], 0.0)

            # Each dma_start stripes over all 16 DMA engines at ~150 GB/s,
            # but a transfer only begins once its issue instruction (~0.75us
            # on-queue) retires — so per-queue issue order must match the
            # conv's consumption order.  Chunks per quarter at col splits
            # t0 / t1-2 / t3-6 / t7-15; sync carries q0+q3, gpsimd carries
            # cw + q1+q2, scalar does the small early loads then evicts.
            CH = P_CORE * QB          # 8192 cols per quarter
            # DMA plan.  Transfers start only after their issue instruction
            # (~0.75us on-queue) retires and each ring serializes its
            # transfers in issue order, so per-queue issue order == arrival
            # order.  sync streams quarters {0,3}, gpsimd {cw,1,2} in
            # t-order with chunks sized to stay ahead of the conv; the
            # scalar ring carries the fused weights (needed by ~11us) and
            # nothing big, so they can't get stuck behind imcol traffic.
            imq_dma(nc.sync, 0, 0, 512)
            imq_dma(nc.sync, 3, 0, 512)
            imq_dma(nc.gpsimd, 1, 0, 512)
            imq_dma(nc.gpsimd, 2, 0, 512)
            nc.gpsimd.dma_start(out=cw_sb[:, :], in_=cw4[:, :])
            # scalar's ring is otherwise idle, so it takes the first-half
            # fused weights plus q3's early chunks (the stalling ones when
            # they queued 6th on sync's ring).
            nc.scalar.dma_start(out=w5_sb[:, 0:320], in_=w5[:, 0:320])
            imq_dma(nc.scalar, 3, 512, 1024)
            imq_dma(nc.scalar, 3, 1024, 2048)
            # preload the ACT function table with a cheap activation so the
            # first real evict doesn't pay the ~2.7us table load.
            nc.scalar.activation(
                out=actwarm[:, :], in_=actwarm[:, :],
                func=mybir.ActivationFunctionType.Relu, bias=0.0, scale=1.0)
            nc.scalar.dma_start(out=w5_sb[:, 320:640], in_=w5[:, 320:640])
            for lo, hi in ((512, 1024), (1024, 2048)):
                imq_dma(nc.sync, 0, lo, hi)
                imq_dma(nc.gpsimd, 1, lo, hi)
                imq_dma(nc.gpsimd, 2, lo, hi)
            for lo, hi in ((2048, 4864), (4864, CH)):
                imq_dma(nc.sync, 0, lo, hi)
                imq_dma(nc.gpsimd, 1, lo, hi)
                imq_dma(nc.gpsimd, 2, lo, hi)
                imq_dma(nc.sync, 3, lo, hi)

            # PE warmup: dummy matmuls open the HAM clock gate (needs ~4us
            # of sustained busy) while the first DMA chunks land; more
            # filler dummies are slotted into the DMA-gated early conv
            # steps below to keep the ramp going instead of idling.
            for _ in range(10):
                nc.tensor.matmul(facc0[:, 0:384], zdummy[:, 0:128],
                                 zdummy[:, :], start=True, stop=True)

            # Relu evictions: pair0 always on ACT (the faster engine) since
            # its completion frees the PSUM tile the next step's pair1 is
            # waiting on — the tight link in the recycle chain; pair1 on
            # DVE.  (POOL has no PSUM access, so only these two can evict.)
            def evict(dst, src, e):
                if e == "act":
                    nc.scalar.activation(
                        out=dst, in_=src,
                        func=mybir.ActivationFunctionType.Relu,
                        bias=0.0, scale=1.0)
                else:
                    nc.vector.tensor_scalar_max(out=dst, in0=src, scalar1=0.0)

            import concourse.bass as bass

            def fused_step(t):
                # fused matmul: 2 positions packed per MM (M=10, N=512).
                # Cross terms (W5[p] x feats0[p+1] etc.) accumulate into the
                # ignored quadrants of the [10, 512] accumulator.
                for dp in range(0, 8, 2):
                    p = 8 * t + dp
                    f = feats0[:, :, p, :]
                    rhs = bass.AP(
                        tensor=f.tensor, offset=f.offset,
                        ap=[f.ap[0], [QB, 2], f.ap[1], f.ap[2]])
                    nc.tensor.matmul(
                        facc0[0:10, :],
                        w5_sb[:, 5 * p:5 * p + 10],
                        rhs,
                        start=(p == 0), stop=(p == P_CORE - 2))

            for t in range(NT):
                # (3+1) PSUM split: quarters 0-2 in a 3-bank double-buffered
                # tile, quarter 3 in the single spare bank.  The q3 tile's
                # small fast evict is the only tight recycle link; the big
                # tile has 2 steps of slack.
                ps3 = convps.tile([128, 3, 512], F32, tag="convps")
                psq = convq3.tile([128, 512], F32, tag="convq3")
                for q in range(NQ):
                    dst = ps3[:, q, :] if q < 3 else psq[:, :]
                    nc.tensor.matmul(
                        dst,
                        cw_sb[32 * q:32 * q + KC2, :],
                        imq[32 * q:32 * q + KC2, 512 * t:512 * (t + 1)],
                        start=True, stop=True,
                        tile_position=(32 * q, 0))
                # relu eviction PSUM -> SBUF (bf16): big+small on opposite
                # engines, alternating per step for capacity balance.
                big, small = ("act", "dve") if t % 2 == 1 else ("dve", "act")
                evict(feats0[:, 0:3, 8 * t:8 * t + 8, :], ps3[:, :, :], big)
                evict(feats0[:, 3, 8 * t:8 * t + 8, :], psq[:, :], small)
                # fused matmuls run two steps behind their evicts so the PE
                # never stalls on eviction completion jitter.
                if t >= 2:
                    fused_step(t - 2)
                    if t <= 3:
                        # filler writes rows 32-95 so it can't clobber the
                        # live fused accumulation in rows 0-9.
                        nc.tensor.matmul(facc0[32:64, 0:384],
                                         zdummy[:, 0:32],
                                         zdummy[:, :], start=True, stop=True)
                else:
                    # filler dummies: keep the PE busy (and the clock ramp
                    # alive) through the DMA-gated first steps.
                    for _ in range(2):
                        nc.tensor.matmul(facc0[:, 0:384], zdummy[:, 0:128],
                                         zdummy[:, :], start=True, stop=True)
            fused_step(NT - 2)
            fused_step(NT - 1)

            # PSUM -> SBUF, then one DMA out.
            nc.vector.tensor_copy(out=out_sb[:, :], in_=facc0[0:10, :])
            nc.sync.dma_start(out=outp[:, :], in_=out_sb[:, :])

    nc.compile()
    return nc


def get_nc():
    if "nc" not in _NC_CACHE:
        _NC_CACHE["nc"] = _build_nc()
    return _NC_CACHE["nc"]


def _host_prep(inputs, conv_w, conv_b, match_w, pat_w, base_w,
               pattern_set_index):
    """Build per-core im2col + fused weight arrays."""
    x = np.ascontiguousarray(np.asarray(inputs, dtype=np.float32))
    # SAME padding for k=3 s=2 on 64 -> pad (0, 1)
    xp = np.zeros((B, HW_IN + 1, HW_IN + 1, CIN), np.float32)
    xp[:, :HW_IN, :HW_IN, :] = x
    s = xp.strides
    win = np.lib.stride_tricks.as_strided(
        xp, shape=(B, P_GRID, P_GRID, 3, 3, CIN),
        strides=(s[0], 2 * s[1], 2 * s[2], s[1], s[2], s[3]))
    # [k, p_global, b]
    imcol = np.ascontiguousarray(win.transpose(3, 4, 5, 1, 2, 0)).reshape(
        KC, P_GRID * P_GRID, B)
    # append the bias ones-row -> K=28
    imcol28 = np.concatenate(
        [imcol, np.ones((1, P_GRID * P_GRID, B), np.float32)], axis=0)
    # [core, q, k, p_local, b_q] -> [8, 4, 28, 8192] bf16
    A = np.ascontiguousarray(
        imcol28.reshape(KC2, N_CORES, P_CORE, NQ, QB).transpose(1, 3, 0, 2, 4)
        .astype(NP_BF16)
    ).reshape(N_CORES, NQ, KC2, P_CORE * QB)

    # conv weights pre-broadcast into the 4 PE row strips, bias in row 27
    cwr = np.asarray(conv_w, np.float32).reshape(KC, COUT)
    cw4 = np.zeros((128, COUT), np.float32)
    for q in range(NQ):
        cw4[32 * q:32 * q + KC, :] = cwr
        cw4[32 * q + KC, :] = np.asarray(conv_b, np.float32)
    cw4 = np.ascontiguousarray(cw4.astype(NP_BF16))

    # fused weight: [p, co, 5] = [base(3) | pat scatter | match/1024]
    base_w3 = np.asarray(base_w, np.float32).reshape(P_GRID * P_GRID, COUT, 3)
    pat_w2 = np.asarray(pat_w, np.float32).reshape(P_GRID * P_GRID, KPAT)
    idx = np.asarray(pattern_set_index).astype(np.int64)
    pw_sc = np.zeros((P_GRID * P_GRID, COUT), np.float32)
    np.add.at(pw_sc,
              (np.repeat(np.arange(P_GRID * P_GRID), KPAT),
               np.tile(idx, P_GRID * P_GRID)),
              pat_w2.ravel())
    W5 = np.zeros((P_GRID * P_GRID, COUT, 5), np.float32)
    W5[:, :, 0:3] = base_w3
    W5[:, :, 3] = pw_sc
    W5[:, :, 4] = np.asarray(match_w, np.float32)[None, :] / float(P_GRID * P_GRID)
    # per-core: [co, p_local, 5] -> [128, 640] bf16
    W5c = np.ascontiguousarray(
        W5.reshape(N_CORES, P_CORE, COUT, 5).transpose(0, 2, 1, 3)
        .astype(NP_BF16)
    ).reshape(N_CORES, COUT, P_CORE * 5)

    return A, cw4, W5c


def kernel(inputs, conv_w, conv_b, match_w, match_b,
           pat_w, pat_b, base_w, base_b, pattern_set_index):
    A, cw4, W5c = _host_prep(inputs, conv_w, conv_b, match_w, pat_w, base_w,
                             pattern_set_index)

    nc = get_nc()
    in_maps = [
        {"imcol": A[c], "cw4": cw4, "w5": W5c[c]}
        for c in range(N_CORES)
    ]
    res = run_bass_kernel_spmd(nc, in_maps, core_ids=list(range(N_CORES)))

    acc = np.zeros((5, B), np.float64)
    for c in range(N_CORES):
        o = res.results[c]["out"].astype(np.float64)  # [10, 2B] packed pairs
        acc += o[0:5, 0:B] + o[5:10, B:2 * B]
    logits = acc.T  # [B, 5]

    # epilogue (host, [256, 5] only)
    base_logits = logits[:, 0:3] + np.asarray(base_b, np.float64)[None, :]
    plogit = logits[:, 3] + float(np.asarray(pat_b).reshape(-1)[0])
    mlogit = logits[:, 4] + float(np.asarray(match_b).reshape(-1)[0])
    p = 1.0 / (1.0 + np.exp(-plogit))
    e = np.exp(base_logits - base_logits.max(axis=1, keepdims=True))
    base = e / e.sum(axis=1, keepdims=True)
    o = (1.0 - p) * 0.5
    cat = np.stack([p, o, o], axis=-1)
    use_pat = (mlogit > 0.0) & (p >= 0.5)
    out = np.where(use_pat[:, None], cat, base)
    return out.astype(np.float32)


# revision 39
# speedup vs baseline: 1.0657x; 1.0657x over previous
"""Trainium2 Bass kernel for nn_PatternBranch (conv3x3/s2+relu -> routed heads).

Strategy
--------
Everything after the conv folds into ONE matmul: with feats0 flattened as
(position p, channel co), the base head, the pattern head (channel-gather
scatter-added over pattern_set_index), and the match head (GAP dot match_w
== sum over (p, co) of feats0 * match_w[co]/1024) concatenate into a single
fused weight W5[p, co, 0:5].  The device then computes, per core:

    conv (im2col K=28 matmul, bias folded in as a ones-row)  ->  PSUM
    -> relu-evict (ACT+DVE+POOL 3-way split) -> feats0 in SBUF
    -> fused matmul  ->  partial logits [5, 256]

Sharding: the 1024 output positions are split 8 ways (128 positions /
core = 4 rows of the 32x32 output grid); every core sees all 256 samples.
Each fused matmul packs TWO positions (M=10, N=512 sample-columns; the
block-cross terms land in ignored PSUM quadrants), and W5 shards cleanly.
Host sums the 8 partial logits and runs the tiny [256,5] epilogue
(sigmoid/softmax/route).

Matmul inputs are bf16 (PE streams 1 col/cycle at 2.4 GHz warm; fp32/fp32r
paths cap at ~1.2 GHz effective).  PSUM accumulation is fp32.  Conv matmuls
are row-tiled (K=28 strips at partitions 32q) so 4 run concurrently; the
conv bias rides along as im2col row 27 = 1.0 with cw row 27 = conv_b, so
the PSUM->SBUF eviction is a bare relu and can rotate over ScalarE,
VectorE and PoolE.  A handful of short dummy matmuls open the HAM clock
gate while the first DMA chunks land; the input DMA is chunked so the
first conv step waits only on the first 1024 columns of each quarter, and
the fused matmuls trail their evictions by two steps so the PE never
stalls.
"""
import sys

for _p in ("/opt/trn_rl_repo", "/root/.axon_site/_ro/trn_rl_repo"):
    if _p not in sys.path:
        sys.path.append(_p)

import numpy as np
import ml_dtypes

import concourse.bacc as bacc
import concourse.mybir as mybir
import concourse.tile as tile
from concourse.bass_utils import run_bass_kernel_spmd

F32 = mybir.dt.float32
BF16 = mybir.dt.bfloat16
NP_BF16 = ml_dtypes.bfloat16

B = 256          # batch
HW_IN = 64       # input spatial
CIN = 3
COUT = 128
KPAT = 32        # gathered channels for pattern head
P_GRID = 32      # output spatial (stride 2)
N_CORES = 8
P_CORE = 128     # positions per core (4 rows of 32)
QB = 64          # samples per quarter
NQ = 4           # quarters (4 x 64 = 256 samples)
KC = 27          # im2col contraction (3*3*3)
KC2 = 28         # + ones-row carrying the conv bias
NT = 16          # time steps: 8 positions x 4 quarters each

_NC_CACHE = {}


def _build_nc():
    """One SPMD program, same for all 8 cores."""
    nc = bacc.Bacc("TRN2", target_bir_lowering=False, debug=False)

    imcol = nc.dram_tensor("imcol", [NQ, KC2, P_CORE * QB], BF16,
                           kind="ExternalInput")
    cw4 = nc.dram_tensor("cw4", [128, COUT], BF16, kind="ExternalInput")
    w5 = nc.dram_tensor("w5", [COUT, P_CORE * 5], BF16, kind="ExternalInput")
    outp = nc.dram_tensor("out", [10, 2 * B], F32, kind="ExternalOutput")

    with tile.TileContext(nc) as tc:
        with tc.tile_pool(name="singles", bufs=1) as singles, \
             tc.tile_pool(name="convps", bufs=2, space="PSUM") as convps, \
             tc.tile_pool(name="convq3", bufs=1, space="PSUM") as convq3, \
             tc.tile_pool(name="faccps", bufs=1, space="PSUM") as faccps:

            cw_sb = singles.tile([128, COUT], BF16)
            imq = singles.tile([128, P_CORE * QB], BF16)
            w5_sb = singles.tile([COUT, P_CORE * 5], BF16)
            actwarm = singles.tile([128, 1], F32)
            zdummy = singles.tile([128, 384], BF16)
            # feats0[co, q, p, b]  (position-major within quarter)
            feats0 = singles.tile([COUT, NQ, P_CORE, QB], BF16)
            out_sb = singles.tile([10, 2 * B], F32)

            facc0 = faccps.tile([128, 2 * B], F32)

            def imq_dma(eng, q, lo, hi):
                eng.dma_start(
                    out=imq[32 * q:32 * q + KC2, lo:hi],
                    in_=imcol[q, :, lo:hi])

            # Vector memsets first so the dummy matmuls and ACT table
            # preload are unblocked immediately.
            nc.vector.memset(actwarm[:, :], 0.0)
            nc.vector.memset(zdummy[:, :], 0.0)

            # Each dma_start stripes over all 16 DMA engines at ~150 GB/s,
            # but a transfer only begins once its issue instruction (~0.75us
            # on-queue) retires — so per-queue issue order must match the
            # conv's consumption order.  Chunks per quarter at col splits
            # t0 / t1-2 / t3-6 / t7-15; sync carries q0+q3, gpsimd carries
            # cw + q1+q2, scalar does the small early loads then evicts.
            CH = P_CORE * QB          # 8192 cols per quarter
            # DMA plan.  Transfers start only after their issue instruction
            # (~0.75us on-queue) retires and each ring serializes its
            # transfers in issue order, so per-queue issue order == arrival
            # order.  sync streams quarters {0,3}, gpsimd {cw,1,2} in
            # t-order with chunks sized to stay ahead of the conv; the
            # scalar ring carries the fused weights (needed by ~11us) and
            # nothing big, so they can't get stuck behind imcol traffic.
            imq_dma(nc.sync, 0, 0, 512)
            imq_dma(nc.sync, 3, 0, 512)
            imq_dma(nc.gpsimd, 1, 0, 512)
            imq_dma(nc.gpsimd, 2, 0, 512)
            nc.gpsimd.dma_start(out=cw_sb[:, :], in_=cw4[:, :])
            nc.scalar.dma_start(out=w5_sb[:, 0:320], in_=w5[:, 0:320])
            # preload the ACT function table with a cheap activation so the
            # first real evict doesn't pay the ~2.7us table load.
            nc.scalar.activation(
                out=actwarm[:, :], in_=actwarm[:, :],
                func=mybir.ActivationFunctionType.Relu, bias=0.0, scale=1.0)
            nc.scalar.dma_start(out=w5_sb[:, 320:640], in_=w5[:, 320:640])
            for lo, hi in ((512, 1024), (1024, 2048), (2048, 4864), (4864, CH)):
                imq_dma(nc.sync, 0, lo, hi)
                imq_dma(nc.gpsimd, 1, lo, hi)
                imq_dma(nc.gpsimd, 2, lo, hi)
                imq_dma(nc.sync, 3, lo, hi)

            # PE warmup: dummy matmuls open the HAM clock gate (needs ~4us
            # of sustained busy) while the first DMA chunks land; more
            # filler dummies are slotted into the DMA-gated early conv
            # steps below to keep the ramp going instead of idling.
            for _ in range(10):
                nc.tensor.matmul(facc0[:, 0:384], zdummy[:, 0:128],
                                 zdummy[:, :], start=True, stop=True)

            # Relu evictions: pair0 always on ACT (the faster engine) since
            # its completion frees the PSUM tile the next step's pair1 is
            # waiting on — the tight link in the recycle chain; pair1 on
            # DVE.  (POOL has no PSUM access, so only these two can evict.)
            def evict(dst, src, e):
                if e == "act":
                    nc.scalar.activation(
                        out=dst, in_=src,
                        func=mybir.ActivationFunctionType.Relu,
                        bias=0.0, scale=1.0)
                else:
                    nc.vector.tensor_scalar_max(out=dst, in0=src, scalar1=0.0)

            import concourse.bass as bass

            def fused_step(t):
                # fused matmul: 2 positions packed per MM (M=10, N=512).
                # Cross terms (W5[p] x feats0[p+1] etc.) accumulate into the
                # ignored quadrants of the [10, 512] accumulator.
                for dp in range(0, 8, 2):
                    p = 8 * t + dp
                    f = feats0[:, :, p, :]
                    rhs = bass.AP(
                        tensor=f.tensor, offset=f.offset,
                        ap=[f.ap[0], [QB, 2], f.ap[1], f.ap[2]])
                    nc.tensor.matmul(
                        facc0[0:10, :],
                        w5_sb[:, 5 * p:5 * p + 10],
                        rhs,
                        start=(p == 0), stop=(p == P_CORE - 2))

            for t in range(NT):
                # (3+1) PSUM split: quarters 0-2 in a 3-bank double-buffered
                # tile, quarter 3 in the single spare bank.  The q3 tile's
                # small fast evict is the only tight recycle link; the big
                # tile has 2 steps of slack.
                ps3 = convps.tile([128, 3, 512], F32, tag="convps")
                psq = convq3.tile([128, 512], F32, tag="convq3")
                for q in range(NQ):
                    dst = ps3[:, q, :] if q < 3 else psq[:, :]
                    nc.tensor.matmul(
                        dst,
                        cw_sb[32 * q:32 * q + KC2, :],
                        imq[32 * q:32 * q + KC2, 512 * t:512 * (t + 1)],
                        start=True, stop=True,
                        tile_position=(32 * q, 0))
                # relu eviction PSUM -> SBUF (bf16): big+small on opposite
                # engines, alternating per step for capacity balance.
                big, small = ("act", "dve") if t % 2 == 1 else ("dve", "act")
                evict(feats0[:, 0:3, 8 * t:8 * t + 8, :], ps3[:, :, :], big)
                evict(feats0[:, 3, 8 * t:8 * t + 8, :], psq[:, :], small)
                # fused matmuls run two steps behind their evicts so the PE
                # never stalls on eviction completion jitter.
                if t >= 2:
                    fused_step(t - 2)
                    if t <= 3:
                        # filler writes rows 32-95 so it can't clobber the
                        # live fused accumulation in rows 0-9.
                        nc.tensor.matmul(facc0[32:64, 0:384],
                                         zdummy[:, 0:32],
                                         zdummy[:, :], start=True, stop=True)
                else:
                    # filler dummies: keep the PE busy (and the clock ramp
                    # alive) through the DMA-gated first steps.
                    for _ in range(2):
                        nc.tensor.matmul(facc0[:, 0:384], zdummy[:, 0:128],
                                         zdummy[:, :], start=True, stop=True)
            fused_step(NT - 2)
            fused_step(NT - 1)

            # PSUM -> SBUF, then one DMA out.
            nc.vector.tensor_copy(out=out_sb[:, :], in_=facc0[0:10, :])
            nc.sync.dma_start(out=outp[:, :], in_=out_sb[:, :])

    nc.compile()
    return nc


def get_nc():
    if "nc" not in _NC_CACHE:
        _NC_CACHE["nc"] = _build_nc()
    return _NC_CACHE["nc"]


def _host_prep(inputs, conv_w, conv_b, match_w, pat_w, base_w,
               pattern_set_index):
    """Build per-core im2col + fused weight arrays."""
    x = np.ascontiguousarray(np.asarray(inputs, dtype=np.float32))
    # SAME padding for k=3 s=2 on 64 -> pad (0, 1)
    xp = np.zeros((B, HW_IN + 1, HW_IN + 1, CIN), np.float32)
    xp[:, :HW_IN, :HW_IN, :] = x
    s = xp.strides
    win = np.lib.stride_tricks.as_strided(
        xp, shape=(B, P_GRID, P_GRID, 3, 3, CIN),
        strides=(s[0], 2 * s[1], 2 * s[2], s[1], s[2], s[3]))
    # [k, p_global, b]
    imcol = np.ascontiguousarray(win.transpose(3, 4, 5, 1, 2, 0)).reshape(
        KC, P_GRID * P_GRID, B)
    # append the bias ones-row -> K=28
    imcol28 = np.concatenate(
        [imcol, np.ones((1, P_GRID * P_GRID, B), np.float32)], axis=0)
    # [core, q, k, p_local, b_q] -> [8, 4, 28, 8192] bf16
    A = np.ascontiguousarray(
        imcol28.reshape(KC2, N_CORES, P_CORE, NQ, QB).transpose(1, 3, 0, 2, 4)
        .astype(NP_BF16)
    ).reshape(N_CORES, NQ, KC2, P_CORE * QB)

    # conv weights pre-broadcast into the 4 PE row strips, bias in row 27
    cwr = np.asarray(conv_w, np.float32).reshape(KC, COUT)
    cw4 = np.zeros((128, COUT), np.float32)
    for q in range(NQ):
        cw4[32 * q:32 * q + KC, :] = cwr
        cw4[32 * q + KC, :] = np.asarray(conv_b, np.float32)
    cw4 = np.ascontiguousarray(cw4.astype(NP_BF16))

    # fused weight: [p, co, 5] = [base(3) | pat scatter | match/1024]
    base_w3 = np.asarray(base_w, np.float32).reshape(P_GRID * P_GRID, COUT, 3)
    pat_w2 = np.asarray(pat_w, np.float32).reshape(P_GRID * P_GRID, KPAT)
    idx = np.asarray(pattern_set_index).astype(np.int64)
    pw_sc = np.zeros((P_GRID * P_GRID, COUT), np.float32)
    np.add.at(pw_sc,
              (np.repeat(np.arange(P_GRID * P_GRID), KPAT),
               np.tile(idx, P_GRID * P_GRID)),
              pat_w2.ravel())
    W5 = np.zeros((P_GRID * P_GRID, COUT, 5), np.float32)
    W5[:, :, 0:3] = base_w3
    W5[:, :, 3] = pw_sc
    W5[:, :, 4] = np.asarray(match_w, np.float32)[None, :] / float(P_GRID * P_GRID)
    # per-core: [co, p_local, 5] -> [128, 640] bf16
    W5c = np.ascontiguousarray(
        W5.reshape(N_CORES, P_CORE, COUT, 5).transpose(0, 2, 1, 3)
        .astype(NP_BF16)
    ).reshape(N_CORES, COUT, P_CORE * 5)

    return A, cw4, W5c


def kernel(inputs, conv_w, conv_b, match_w, match_b,
           pat_w, pat_b, base_w, base_b, pattern_set_index):
    A, cw4, W5c = _host_prep(inputs, conv_w, conv_b, match_w, pat_w, base_w,
                             pattern_set_index)

    nc = get_nc()
    in_maps = [
        {"imcol": A[c], "cw4": cw4, "w5": W5c[c]}
        for c in range(N_CORES)
    ]
    res = run_bass_kernel_spmd(nc, in_maps, core_ids=list(range(N_CORES)))

    acc = np.zeros((5, B), np.float64)
    for c in range(N_CORES):
        o = res.results[c]["out"].astype(np.float64)  # [10, 2B] packed pairs
        acc += o[0:5, 0:B] + o[5:10, B:2 * B]
    logits = acc.T  # [B, 5]

    # epilogue (host, [256, 5] only)
    base_logits = logits[:, 0:3] + np.asarray(base_b, np.float64)[None, :]
    plogit = logits[:, 3] + float(np.asarray(pat_b).reshape(-1)[0])
    mlogit = logits[:, 4] + float(np.asarray(match_b).reshape(-1)[0])
    p = 1.0 / (1.0 + np.exp(-plogit))
    e = np.exp(base_logits - base_logits.max(axis=1, keepdims=True))
    base = e / e.sum(axis=1, keepdims=True)
    o = (1.0 - p) * 0.5
    cat = np.stack([p, o, o], axis=-1)
    use_pat = (mlogit > 0.0) & (p >= 0.5)
    out = np.where(use_pat[:, None], cat, base)
    return out.astype(np.float32)
